# revision 1
# baseline (speedup 1.0000x reference)
"""DeepWalk random-walk kernel for 8 Trainium2 NeuronCores.

Problem (hardcoded from spec): CSR graph with N=100000 nodes, fixed
out-degree 16 (indptr = arange(N+1)*16), indices[1.6M] int32 random,
rand_vals [10, 100000, 80] f32. Output walks [10, 100000, 80] f32 where
walks[w,n,t] = node id at step t (walks never die: deg==16 for all nodes).

Recurrence per walk: v_{t+1} = indices[v_t*16 + floor(u_t*16)], record v_t.

Strategy: shard the 100000 start nodes across 8 cores (12500 each ->
125000 walks/core). Walks live in SBUF as [128, 977] f32 slots
(walk j -> partition j%128, column j//128). Per step:
  - DVE computes q = v*16 + floor(u*16) exactly in f32 (floor built from
    round-to-nearest int cast + is_gt correction), casts to int32.
  - Pool engine issues 977 per-column indirect DMAs (SWDGE vector-indirect
    gather): out[p,k] = table_f32[q[p,k]] -- the table is the neighbor
    array pre-converted to f32 in DRAM (values < 2^24 so exact).
  - Sync engine streams u in (double-buffered) and records v out to DRAM.
80 steps via a hardware Fori loop (2-step unrolled body, ping-pong bufs).
Gathers, DVE index math, and the record DMA are all split into two
semaphore halves per step so half B's SDMA drain overlaps the next step's
half-A work; the remaining ~2.9 ms/step is the SDMA random-read latency
floor (128 4B DRAM reads/instr over 16 engines at ~360 ns each).
Host pre/post: pure layout reshapes + int->f32 table conversion.
"""

import sys

sys.path.insert(0, "/opt/trn_rl_repo")

import numpy as np

import concourse.bacc as bacc
import concourse.bass as bass
import concourse.mybir as mybir
from concourse import bass_utils
from concourse.bass import ds

N_NODES = 100000
DEGREE = 16
WALKS_PER_VERTEX = 10
WALK_LENGTH = 80
NCORES = 8
NSH = N_NODES // NCORES          # nodes per core
WALKS = WALKS_PER_VERTEX * NSH   # walks per core
P = 128
COLS = (WALKS + P - 1) // P      # 977
PAD = P * COLS                   # 125056
COLS_A = COLS // 2               # first-half columns (488)
COLS_B = COLS - COLS_A           # second half (489)
GINC_A = COLS_A * 16
GINC_B = COLS_B * 16

_cache = {}


def _build(n_steps):
    f32 = mybir.dt.float32
    i32 = mybir.dt.int32
    nc = bacc.Bacc("TRN2", debug=False)

    tab_d = nc.dram_tensor("tab", [N_NODES * DEGREE, 1], f32, kind="ExternalInput")
    u_d = nc.dram_tensor("u", [(n_steps + 2) * P, COLS], f32, kind="ExternalInput")
    v0_d = nc.dram_tensor("v0", [P, COLS], f32, kind="ExternalInput")
    w_d = nc.dram_tensor("walks", [n_steps * P, COLS], f32, kind="ExternalOutput")

    v_bufs = [nc.alloc_sbuf_tensor(f"vb{s}", [P, COLS], f32).ap() for s in (0, 1)]
    u_bufs = [nc.alloc_sbuf_tensor(f"ub{s}", [P, COLS], f32).ap() for s in (0, 1)]
    t1 = nc.alloc_sbuf_tensor("t1", [P, COLS], f32).ap()
    fl = nc.alloc_sbuf_tensor("fl", [P, COLS], f32).ap()
    gt = nc.alloc_sbuf_tensor("gt", [P, COLS], f32).ap()
    qf = nc.alloc_sbuf_tensor("qf", [P, COLS], f32).ap()
    ri = nc.alloc_sbuf_tensor("ri", [P, COLS], i32).ap()
    qi = nc.alloc_sbuf_tensor("qi", [P, COLS], i32).ap()

    uin_sem = nc.alloc_semaphore()
    dveA_sem = nc.alloc_semaphore()
    dveB_sem = nc.alloc_semaphore()
    gA_sem = nc.alloc_semaphore()
    gB_sem = nc.alloc_semaphore()
    outA_sem = nc.alloc_semaphore()
    outB_sem = nc.alloc_semaphore()

    # prologue: load v0 and u_0
    nc.sync.dma_start(v_bufs[0][:], v0_d.ap()[:, :]).then_inc(uin_sem, 16)
    nc.sync.dma_start(u_bufs[0][:], u_d.ap()[0:P, :]).then_inc(uin_sem, 16)
    nc.sync.wait_ge(uin_sem, 32)
    nc.vector.wait_ge(uin_sem, 32)

    ALL = [mybir.EngineType.Pool, mybir.EngineType.DVE, mybir.EngineType.SP]

    def step_body(i, s):
        # t = 2*i + s (ScalarValue expression); constants folded per s
        cur = v_bufs[s]
        nxt = v_bufs[1 - s]
        ucur = u_bufs[s]
        unxt = u_bufs[1 - s]

        t_gA = i * (2 * GINC_A) + s * GINC_A     # 16*COLS_A*t
        t_gB = i * (2 * GINC_B) + s * GINC_B
        t_1 = i * 2 + s                          # t
        t_16 = i * 32 + s * 16                   # 16*t
        row0 = i * (2 * P) + s * P               # 128*t

        # --- sync engine: record v_t (split in halves so gather-gen of the
        # next step only depends on its own half's record), prefetch u ---
        nc.sync.wait_ge(gA_sem, t_gA)
        nc.sync.dma_start(w_d.ap()[ds(row0, P), 0:COLS_A],
                          cur[:, 0:COLS_A]).then_inc(outA_sem, 16)
        nc.sync.wait_ge(gB_sem, t_gB)
        nc.sync.dma_start(w_d.ap()[ds(row0, P), COLS_A:COLS],
                          cur[:, COLS_A:COLS]).then_inc(outB_sem, 16)
        nc.sync.wait_ge(dveB_sem, t_1)
        nc.sync.dma_start(unxt[:], u_d.ap()[ds(row0 + P, P), :]).then_inc(uin_sem, 16)

        # --- DVE: q = v*16 + floor(u*16), in halves so gathers can start
        # on half A while half B of the previous step still drains ---
        nc.vector.wait_ge(uin_sem, i * 32 + s * 16 + 32)   # u_t present
        def half(sl, done_sem, done_val, inc_sem):
            nc.vector.wait_ge(done_sem, done_val)          # v_t half present; q half free
            nc.vector.tensor_scalar_mul(t1[:, sl], ucur[:, sl], 16.0)
            nc.vector.tensor_copy(ri[:, sl], t1[:, sl])    # round-to-nearest
            nc.vector.tensor_copy(fl[:, sl], ri[:, sl])    # back to f32 (exact)
            nc.vector.tensor_tensor(gt[:, sl], fl[:, sl], t1[:, sl], op=mybir.AluOpType.is_gt)
            nc.vector.tensor_tensor(fl[:, sl], fl[:, sl], gt[:, sl], op=mybir.AluOpType.subtract)
            nc.vector.tensor_scalar_mul(qf[:, sl], cur[:, sl], 16.0)
            nc.vector.tensor_tensor(qf[:, sl], qf[:, sl], fl[:, sl], op=mybir.AluOpType.add)
            nc.vector.tensor_copy(qi[:, sl], qf[:, sl]).then_inc(inc_sem, 1)
        half(slice(0, COLS_A), gA_sem, t_gA, dveA_sem)
        half(slice(COLS_A, COLS), gB_sem, t_gB, dveB_sem)

        # --- Pool: per-column indirect gathers, half A then half B ---
        nc.gpsimd.wait_ge(outA_sem, t_16)
        nc.gpsimd.wait_ge(dveA_sem, t_1 + 1)
        for k in range(COLS_A):
            nc.gpsimd.indirect_dma_start(
                out=nxt[:, k:k + 1],
                out_offset=None,
                in_=tab_d.ap()[:, :],
                in_offset=bass.IndirectOffsetOnAxis(ap=qi[:, k:k + 1], axis=0),
            ).then_inc(gA_sem, 16)
        nc.gpsimd.wait_ge(outB_sem, t_16)
        nc.gpsimd.wait_ge(dveB_sem, t_1 + 1)
        for k in range(COLS_A, COLS):
            nc.gpsimd.indirect_dma_start(
                out=nxt[:, k:k + 1],
                out_offset=None,
                in_=tab_d.ap()[:, :],
                in_offset=bass.IndirectOffsetOnAxis(ap=qi[:, k:k + 1], axis=0),
            ).then_inc(gB_sem, 16)

    assert n_steps % 2 == 0
    with nc.Fori(0, n_steps // 2, engines=ALL) as i:
        step_body(i, 0)
        step_body(i, 1)

    nc.sync.wait_ge(outA_sem, 16 * n_steps)
    nc.sync.wait_ge(outB_sem, 16 * n_steps)
    nc.sync.wait_ge(gA_sem, GINC_A * n_steps)
    nc.sync.wait_ge(gB_sem, GINC_B * n_steps)
    nc.all_engine_barrier()
    nc.finalize()
    return nc


def _get_nc(n_steps):
    if n_steps not in _cache:
        _cache[n_steps] = _build(n_steps)
    return _cache[n_steps]


def kernel(indptr, indices, rand_vals):
    indptr = np.asarray(indptr)
    indices = np.asarray(indices)
    rand_vals = np.asarray(rand_vals)
    W, N, L = rand_vals.shape
    assert (W, N) == (WALKS_PER_VERTEX, N_NODES) and L % 2 == 0
    # the kernel exploits the fixed out-degree structure
    assert np.array_equal(indptr, (np.arange(N + 1) * DEGREE).astype(np.int32))

    tab = np.ascontiguousarray(indices.astype(np.float32).reshape(-1, 1))

    in_maps = []
    for c in range(NCORES):
        sl = rand_vals[:, c * NSH:(c + 1) * NSH, :]           # [W, NSH, L]
        U = sl.reshape(WALKS, L)                               # walk-major j = w*NSH+n
        U_pad = np.zeros((PAD, L), np.float32)
        U_pad[:WALKS] = U
        # u_pre[t, p, f] = U_pad[f*128 + p, t]
        u_pre = U_pad.T.reshape(L, COLS, P).swapaxes(1, 2)     # [L, P, COLS]
        u_full = np.zeros(((L + 2) * P, COLS), np.float32)
        u_full[:L * P] = u_pre.reshape(L * P, COLS)

        j = np.arange(PAD)
        v0 = np.where(j < WALKS, c * NSH + (j % NSH), 0).astype(np.float32)
        v0 = v0.reshape(COLS, P).T.copy()                      # [P, COLS]

        in_maps.append({"tab": tab, "u": np.ascontiguousarray(u_full), "v0": v0})

    nc = _get_nc(L)
    res = bass_utils.run_bass_kernel_spmd(nc, in_maps, core_ids=list(range(NCORES)))

    out = np.empty((W, N, L), np.float32)
    for c in range(NCORES):
        w_t = res.results[c]["walks"]                          # [L*P, COLS]
        Wc = w_t.reshape(L, P, COLS).swapaxes(1, 2).reshape(L, PAD)[:, :WALKS]
        out[:, c * NSH:(c + 1) * NSH, :] = Wc.T.reshape(W, NSH, L)
    return out



# revision 2
# speedup vs baseline: 1.0698x; 1.0698x over previous
"""DeepWalk random-walk kernel for 8 Trainium2 NeuronCores.

Problem (hardcoded from spec): CSR graph with N=100000 nodes, fixed
out-degree 16 (indptr = arange(N+1)*16), indices[1.6M] int32 random,
rand_vals [10, 100000, 80] f32. Output walks [10, 100000, 80] f32 where
walks[w,n,t] = node id at step t (walks never die: deg==16 for all nodes).

Recurrence per walk: v_{t+1} = indices[v_t*16 + floor(u_t*16)], record v_t.

Strategy: shard the 100000 start nodes across 8 cores (12500 each ->
125000 walks/core). Walks live in SBUF as [128, 977] f32 slots
(walk j -> partition j%128, column j//128). Per step:
  - DVE computes q = v*16 + floor(u*16) exactly in f32 (floor built from
    round-to-nearest int cast + is_gt correction), casts to int32.
  - Pool engine issues 977 per-column indirect DMAs (SWDGE vector-indirect
    gather): out[p,k] = table_f32[q[p,k]] -- the table is the neighbor
    array pre-converted to f32 in DRAM (values < 2^24 so exact).
  - Sync engine streams u in (double-buffered) and records v out to DRAM.
80 steps via a hardware Fori loop (2-step unrolled body, ping-pong bufs).
Gathers, DVE index math, and the record DMA are all split into two
semaphore halves per step so half B's SDMA drain overlaps the next step's
half-A work; the remaining ~2.9 ms/step is the SDMA random-read latency
floor (128 4B DRAM reads/instr over 16 engines at ~360 ns each).
Host pre/post: pure layout reshapes + int->f32 table conversion.
"""

import sys

sys.path.insert(0, "/opt/trn_rl_repo")

import numpy as np

import concourse.bacc as bacc
import concourse.bass as bass
import concourse.mybir as mybir
from concourse import bass_utils
from concourse.bass import ds

N_NODES = 100000
DEGREE = 16
WALKS_PER_VERTEX = 10
WALK_LENGTH = 80
NCORES = 8
NSH = N_NODES // NCORES          # nodes per core
WALKS = WALKS_PER_VERTEX * NSH   # walks per core
P = 128
COLS = (WALKS + P - 1) // P      # 977
PAD = P * COLS                   # 125056
COLS_A = COLS // 2               # first-half columns (488)
COLS_B = COLS - COLS_A           # second half (489)
GINC_A = COLS_A * 16
GINC_B = COLS_B * 16

_cache = {}


def _indirect_q(g, out, in_, offset_ap, queue):
    out_l = g.lower_ap_dma(out, for_indirect_dma=True)
    in_l = g.lower_ap_dma(in_, for_indirect_dma=True)
    off_l = g.lower_ap_dma(offset_ap)
    in_l.append(off_l[0])
    shape = in_.shape
    coef = 1
    for i in range(1, len(shape)):
        coef *= shape[i]
    in_l[0].dynamic_ap_info = mybir.DynamicAccessPatternInfo(
        c=0,
        actual_ap=out.ap,
        indirect_dim_max_index=shape[0],
        offset_expr=[
            mybir.DynamicAccessPatternOffsetExpr(
                coef=coef,
                aff_expr=mybir.DynamicAccessPatternOffsetExprAffExpr(
                    kind="IndirectArgId", arg_id=1
                ),
            )
        ],
    )
    return g.add_instruction(
        mybir.InstDMACopy(
            name=g.bass.get_next_instruction_name(),
            queue=queue,
            mode="Copy",
            ins=in_l,
            outs=out_l,
            oob_is_err=True,
            cce_op=mybir.AluOpType.bypass,
        )
    )



def _build(n_steps):
    f32 = mybir.dt.float32
    i32 = mybir.dt.int32
    nc = bacc.Bacc("TRN2", debug=False, num_swdge_queues=2)

    tab_d = nc.dram_tensor("tab", [N_NODES * DEGREE, 1], f32, kind="ExternalInput")
    u_d = nc.dram_tensor("u", [(WALK_LENGTH + 2) * P, COLS], f32, kind="ExternalInput")
    v0_d = nc.dram_tensor("v0", [P, COLS], f32, kind="ExternalInput")
    w_d = nc.dram_tensor("walks", [WALK_LENGTH * P, COLS], f32, kind="ExternalOutput")

    v_bufs = [nc.alloc_sbuf_tensor(f"vb{s}", [P, COLS], f32).ap() for s in (0, 1)]
    u_bufs = [nc.alloc_sbuf_tensor(f"ub{s}", [P, COLS], f32).ap() for s in (0, 1)]
    t1 = nc.alloc_sbuf_tensor("t1", [P, COLS], f32).ap()
    fl = nc.alloc_sbuf_tensor("fl", [P, COLS], f32).ap()
    gt = nc.alloc_sbuf_tensor("gt", [P, COLS], f32).ap()
    qf = nc.alloc_sbuf_tensor("qf", [P, COLS], f32).ap()
    ri = nc.alloc_sbuf_tensor("ri", [P, COLS], i32).ap()
    qi = nc.alloc_sbuf_tensor("qi", [P, COLS], i32).ap()

    uin_sem = nc.alloc_semaphore()
    dveA_sem = nc.alloc_semaphore()
    dveB_sem = nc.alloc_semaphore()
    gA_sem = nc.alloc_semaphore()
    gB_sem = nc.alloc_semaphore()
    outA_sem = nc.alloc_semaphore()
    outB_sem = nc.alloc_semaphore()

    # prologue: load v0 and u_0
    nc.sync.dma_start(v_bufs[0][:], v0_d.ap()[:, :]).then_inc(uin_sem, 16)
    nc.sync.dma_start(u_bufs[0][:], u_d.ap()[0:P, :]).then_inc(uin_sem, 16)
    nc.sync.wait_ge(uin_sem, 32)
    nc.vector.wait_ge(uin_sem, 32)

    ALL = [mybir.EngineType.Pool, mybir.EngineType.DVE, mybir.EngineType.SP]

    def step_body(i, s):
        # t = 2*i + s (ScalarValue expression); constants folded per s
        cur = v_bufs[s]
        nxt = v_bufs[1 - s]
        ucur = u_bufs[s]
        unxt = u_bufs[1 - s]

        t_gA = i * (2 * GINC_A) + s * GINC_A     # 16*COLS_A*t
        t_gB = i * (2 * GINC_B) + s * GINC_B
        t_1 = i * 2 + s                          # t
        t_16 = i * 32 + s * 16                   # 16*t
        row0 = i * (2 * P) + s * P               # 128*t

        # --- sync engine: record v_t (split in halves so gather-gen of the
        # next step only depends on its own half's record), prefetch u ---
        nc.sync.wait_ge(gA_sem, t_gA)
        nc.sync.dma_start(w_d.ap()[ds(row0, P), 0:COLS_A],
                          cur[:, 0:COLS_A]).then_inc(outA_sem, 16)
        nc.sync.wait_ge(gB_sem, t_gB)
        nc.sync.dma_start(w_d.ap()[ds(row0, P), COLS_A:COLS],
                          cur[:, COLS_A:COLS]).then_inc(outB_sem, 16)
        nc.sync.wait_ge(dveB_sem, t_1)
        nc.sync.dma_start(unxt[:], u_d.ap()[ds(row0 + P, P), :]).then_inc(uin_sem, 16)

        # --- DVE: q = v*16 + floor(u*16), in halves so gathers can start
        # on half A while half B of the previous step still drains ---
        nc.vector.wait_ge(uin_sem, i * 32 + s * 16 + 32)   # u_t present
        def half(sl, done_sem, done_val, inc_sem):
            nc.vector.wait_ge(done_sem, done_val)          # v_t half present; q half free
            nc.vector.tensor_scalar_mul(t1[:, sl], ucur[:, sl], 16.0)
            nc.vector.tensor_copy(ri[:, sl], t1[:, sl])    # round-to-nearest
            nc.vector.tensor_copy(fl[:, sl], ri[:, sl])    # back to f32 (exact)
            nc.vector.tensor_tensor(gt[:, sl], fl[:, sl], t1[:, sl], op=mybir.AluOpType.is_gt)
            nc.vector.tensor_tensor(fl[:, sl], fl[:, sl], gt[:, sl], op=mybir.AluOpType.subtract)
            nc.vector.tensor_scalar_mul(qf[:, sl], cur[:, sl], 16.0)
            nc.vector.tensor_tensor(qf[:, sl], qf[:, sl], fl[:, sl], op=mybir.AluOpType.add)
            nc.vector.tensor_copy(qi[:, sl], qf[:, sl]).then_inc(inc_sem, 1)
        half(slice(0, COLS_A), gA_sem, t_gA, dveA_sem)
        half(slice(COLS_A, COLS), gB_sem, t_gB, dveB_sem)

        # --- Pool: per-column indirect gathers, half A then half B ---
        nc.gpsimd.wait_ge(outA_sem, t_16)
        nc.gpsimd.wait_ge(dveA_sem, t_1 + 1)
        for k in range(COLS_A):
            _indirect_q(nc.gpsimd, nxt[:, k:k + 1], tab_d.ap()[:, :],
                        qi[:, k:k + 1],
                        "qPoolDynamic" + ("1" if k % 2 else "")
                        ).then_inc(gA_sem, 16)
        nc.gpsimd.wait_ge(outB_sem, t_16)
        nc.gpsimd.wait_ge(dveB_sem, t_1 + 1)
        for k in range(COLS_A, COLS):
            _indirect_q(nc.gpsimd, nxt[:, k:k + 1], tab_d.ap()[:, :],
                        qi[:, k:k + 1],
                        "qPoolDynamic" + ("1" if k % 2 else "")
                        ).then_inc(gB_sem, 16)

    assert n_steps % 2 == 0
    with nc.Fori(0, n_steps // 2, engines=ALL) as i:
        step_body(i, 0)
        step_body(i, 1)

    nc.sync.wait_ge(outA_sem, 16 * n_steps)
    nc.sync.wait_ge(outB_sem, 16 * n_steps)
    nc.sync.wait_ge(gA_sem, GINC_A * n_steps)
    nc.sync.wait_ge(gB_sem, GINC_B * n_steps)
    nc.all_engine_barrier()
    nc.finalize()
    return nc


def _get_nc(n_steps):
    if n_steps not in _cache:
        _cache[n_steps] = _build(n_steps)
    return _cache[n_steps]


def kernel(indptr, indices, rand_vals):
    indptr = np.asarray(indptr)
    indices = np.asarray(indices)
    rand_vals = np.asarray(rand_vals)
    W, N, L = rand_vals.shape
    assert (W, N) == (WALKS_PER_VERTEX, N_NODES) and L % 2 == 0
    # the kernel exploits the fixed out-degree structure
    assert np.array_equal(indptr, (np.arange(N + 1) * DEGREE).astype(np.int32))

    tab = np.ascontiguousarray(indices.astype(np.float32).reshape(-1, 1))

    in_maps = []
    for c in range(NCORES):
        sl = rand_vals[:, c * NSH:(c + 1) * NSH, :]           # [W, NSH, L]
        U = sl.reshape(WALKS, L)                               # walk-major j = w*NSH+n
        U_pad = np.zeros((PAD, L), np.float32)
        U_pad[:WALKS] = U
        # u_pre[t, p, f] = U_pad[f*128 + p, t]
        u_pre = U_pad.T.reshape(L, COLS, P).swapaxes(1, 2)     # [L, P, COLS]
        u_full = np.zeros(((L + 2) * P, COLS), np.float32)
        u_full[:L * P] = u_pre.reshape(L * P, COLS)

        j = np.arange(PAD)
        v0 = np.where(j < WALKS, c * NSH + (j % NSH), 0).astype(np.float32)
        v0 = v0.reshape(COLS, P).T.copy()                      # [P, COLS]

        in_maps.append({"tab": tab, "u": np.ascontiguousarray(u_full), "v0": v0})

    nc = _get_nc(L)
    res = bass_utils.run_bass_kernel_spmd(nc, in_maps, core_ids=list(range(NCORES)))

    out = np.empty((W, N, L), np.float32)
    for c in range(NCORES):
        w_t = res.results[c]["walks"]                          # [L*P, COLS]
        Wc = w_t.reshape(L, P, COLS).swapaxes(1, 2).reshape(L, PAD)[:, :WALKS]
        out[:, c * NSH:(c + 1) * NSH, :] = Wc.T.reshape(W, NSH, L)
    return out



def _prep_in_maps(indices, rand_vals):
    L = rand_vals.shape[2]
    tab = np.ascontiguousarray(indices.astype(np.float32).reshape(-1, 1))
    in_maps = []
    for c in range(NCORES):
        sl = rand_vals[:, c * NSH:(c + 1) * NSH, :]
        U = sl.reshape(WALKS, L)
        U_pad = np.zeros((PAD, L), np.float32)
        U_pad[:WALKS] = U
        u_pre = U_pad.T.reshape(L, COLS, P).swapaxes(1, 2)
        u_full = np.zeros(((L + 2) * P, COLS), np.float32)
        u_full[:L * P] = u_pre.reshape(L * P, COLS)
        j = np.arange(PAD)
        v0 = np.where(j < WALKS, c * NSH + (j % NSH), 0).astype(np.float32)
        v0 = v0.reshape(COLS, P).T.copy()
        in_maps.append({"tab": tab, "u": np.ascontiguousarray(u_full), "v0": v0})
    return in_maps


def timing_pair(inputs):
    indices = np.asarray(inputs["indices"])
    rand_vals = np.asarray(inputs["rand_vals"])
    in_maps = _prep_in_maps(indices, rand_vals)
    return in_maps, _get_nc(WALK_LENGTH), _get_nc(2)
